# revision 6
# baseline (speedup 1.0000x reference)
"""AttnBlock++ (GroupNorm + single-head 1x1-conv attention + residual) on 8 TRN2 NeuronCores.

Sharding: 8 cores = 4 samples x 2 query-halves. Each core:
  - holds the full sample x[b] [256, 4096] (GroupNorm stats + K/V need all pixels)
  - computes q only for its half of the 4096 pixels (2048 columns)
  - attention S^T = k.T q in [m, n] layout (m = key pixel on partitions), exp on ACT,
    AV + softmax denominator accumulated in PSUM, normalization folded into the
    output projection epilogue.
GroupNorm is folded into the QKV projection weights (W' = A_c * W, bias fold),
so the normalized activation h is never materialized.
All heavy matmuls run in float32r (full-rate fp32 on the PE at N>=256).
"""
import sys

for _p in ("/opt/trn_rl_repo",):
    if _p not in sys.path:
        sys.path.append(_p)

import math
import numpy as np

import concourse.bacc as bacc
import concourse.tile as tile
from concourse import mybir
from concourse import bass_utils

B, C, HW = 4, 256, 4096
NH = HW // 2          # query pixels per core
P = 128
GSIZE = 8             # channels per group
EPS = 1e-5
F32 = mybir.dt.float32
F32R = mybir.dt.float32r
RS2 = 1.0 / math.sqrt(2.0)
SQ2 = math.sqrt(2.0)
AluOp = mybir.AluOpType
Act = mybir.ActivationFunctionType


def _build():
    nc = bacc.Bacc("TRN2", target_bir_lowering=False, debug=False)

    dx = nc.dram_tensor("xf", [C, HW], F32R, kind="ExternalInput").ap()
    dxq = nc.dram_tensor("xq", [C, NH], F32R, kind="ExternalInput").ap()
    dwq = nc.dram_tensor("wq", [C, C], F32, kind="ExternalInput").ap()
    dwk = nc.dram_tensor("wk", [C, C], F32, kind="ExternalInput").ap()
    dwv = nc.dram_tensor("wv", [C, C], F32, kind="ExternalInput").ap()
    dwp = nc.dram_tensor("wp", [C, C], F32R, kind="ExternalInput").ap()
    dgw = nc.dram_tensor("gnw", [C], F32, kind="ExternalInput").ap()
    dgb = nc.dram_tensor("gnb", [C], F32, kind="ExternalInput").ap()
    dbq = nc.dram_tensor("bq", [C], F32, kind="ExternalInput").ap()
    dbv = nc.dram_tensor("bv", [C], F32, kind="ExternalInput").ap()
    dbp = nc.dram_tensor("bp", [C], F32, kind="ExternalInput").ap()
    dgm = nc.dram_tensor("gmat", [P, 16], F32, kind="ExternalInput").ap()
    dgmt = nc.dram_tensor("gmatT", [16, P], F32, kind="ExternalInput").ap()
    dones = nc.dram_tensor("ones", [P, 1], F32R, kind="ExternalInput").ap()
    dout = nc.dram_tensor("out", [C, NH], F32, kind="ExternalOutput").ap()

    with tile.TileContext(nc) as tc:
        with (
            tc.tile_pool(name="persist", bufs=1) as pp,
            tc.tile_pool(name="expp", bufs=3) as expp,
            tc.tile_pool(name="avp", bufs=3) as avp,
            tc.tile_pool(name="outp", bufs=2) as outp,
            tc.tile_pool(name="rbp", bufs=1) as rbp,
            tc.tile_pool(name="ps_s", bufs=3, space="PSUM") as ps_s,
            tc.tile_pool(name="ps_av", bufs=1, space="PSUM") as ps_av,
            tc.tile_pool(name="ps_db", bufs=1, space="PSUM") as ps_db,
            tc.tile_pool(name="ps_misc", bufs=2, space="PSUM") as ps_misc,
        ):
            # ---- persistent SBUF ----
            xf_t = pp.tile([P, 2, HW], F32R, tag="xf")      # x sample, [c-half, pixel]
            xqs_t = pp.tile([P, 2, NH], F32R, tag="xqs")    # xq / sqrt(2)
            k_t = pp.tile([P, 2, HW], F32R, tag="k")        # [d-half, m]
            q_t = pp.tile([P, 2, NH], F32R, tag="q")        # [d-half, n]
            vt_t = pp.tile([P, 32, C], F32R, tag="vt")      # [m-tile, d]
            wraw = {
                "q": pp.tile([P, 2, C], F32, name="wrawq", tag="wrawq"),
                "k": pp.tile([P, 2, C], F32, name="wrawk", tag="wrawk"),
                "v": pp.tile([P, 2, C], F32, name="wrawv", tag="wrawv"),
            }
            wp_t = pp.tile([P, 2, C], F32R, tag="wp")
            wr = {
                "q": pp.tile([P, 2, C], F32R, name="wrq", tag="wrq"),
                "k": pp.tile([P, 2, C], F32R, name="wrk", tag="wrk"),
                "v": pp.tile([P, 2, C], F32R, name="wrv", tag="wrv"),
            }
            ones_t = pp.tile([P, 1], F32R, tag="ones")
            gmat_t = pp.tile([P, 16], F32, tag="gmat")
            gmatT_t = pp.tile([16, P], F32, tag="gmatT")
            gw_t = pp.tile([P, 2], F32, tag="gw")
            gb_t = pp.tile([P, 2], F32, tag="gb")
            bq_t = pp.tile([P, 2], F32, tag="bq")
            bv_t = pp.tile([P, 2], F32, tag="bv")
            bp_t = pp.tile([P, 2], F32, tag="bp")
            stat_t = pp.tile([P, 2, 2], F32, tag="stat")    # per c-half: (mean_c, E[x^2]_c)
            bst_t = pp.tile([P, 2, 8, 6], F32, tag="bst")      # bn_stats subgroup stats
            mvc_t = pp.tile([P, 2, 2], F32, tag="mvc")      # per-channel (mean, var)
            s16_t = pp.tile([16, 2, 2], F32, tag="s16")
            mv_t = pp.tile([P, 2, 2], F32, tag="mv")
            t1_t = pp.tile([P, 2], F32, tag="t1")
            t2_t = pp.tile([P, 2], F32, tag="t2")
            t3_t = pp.tile([P, 2], F32, tag="t3")
            sr_t = pp.tile([P, 2], F32, tag="sr")
            ve_t = pp.tile([P, 2], F32, tag="ve")
            r0_t = pp.tile([P, 2], F32, tag="r0")
            rn_t = pp.tile([P, 2], F32, tag="rn")
            A_t = pp.tile([P, 2], F32, tag="A")
            Aq_t = pp.tile([P, 2], F32, tag="Aq")
            nB_t = pp.tile([P, 2], F32, tag="nB")
            bqs_t = pp.tile([P, 2], F32, tag="bqs")
            bps_t = pp.tile([P, 2], F32, tag="bps")
            biasq_t = pp.tile([P, 2], F32, tag="biasq")
            bvp_t = pp.tile([P, 2], F32, tag="bvp")
            beta_t = pp.tile([P, 2], F32, tag="beta")
            eps_t = pp.tile([P, 1], F32, tag="eps")

            # ---- input DMAs ----
            dxr = dx.rearrange("(i p) n -> p i n", p=P)
            for sg in range(8):
                ss = slice(sg * 512, (sg + 1) * 512)
                nc.sync.dma_start(out=xf_t[:, :, ss], in_=dxr[:, :, ss])
            nc.gpsimd.dma_start(out=xqs_t[:], in_=dxq.rearrange("(i p) n -> p i n", p=P))
            nc.gpsimd.dma_start(out=wraw["q"][:], in_=dwq.rearrange("(i p) d -> p i d", p=P))
            nc.gpsimd.dma_start(out=wraw["k"][:], in_=dwk.rearrange("(i p) d -> p i d", p=P))
            nc.gpsimd.dma_start(out=wraw["v"][:], in_=dwv.rearrange("(i p) d -> p i d", p=P))
            nc.gpsimd.dma_start(out=wp_t[:], in_=dwp.rearrange("(i p) d -> p i d", p=P))
            nc.gpsimd.dma_start(out=ones_t[:], in_=dones[:, :])
            nc.gpsimd.dma_start(out=gmat_t[:], in_=dgm[:, :])
            nc.gpsimd.dma_start(out=gmatT_t[:], in_=dgmt[:, :])
            for dst, src in ((gw_t, dgw), (gb_t, dgb), (bq_t, dbq), (bv_t, dbv), (bp_t, dbp)):
                nc.gpsimd.dma_start(out=dst[:], in_=src.rearrange("(j p) -> p j", p=P))

            nc.vector.memset(eps_t[:], EPS)

            # xq pre-scale by 1/sqrt(2), in place (rounds to f32r)
            for i in range(2):
                nc.vector.tensor_scalar_mul(xqs_t[:, i, :], xqs_t[:, i, :], RS2)

            # ---- GroupNorm stats: per-channel mean/var via bn_stats ----
            xr = {i: xf_t[:, i, :].bitcast(F32).rearrange("p (s f) -> p s f", f=512)
                  for i in range(2)}
            for sg in range(8):
                for i in range(2):
                    nc.vector.bn_stats(out=bst_t[:, i, sg, :], in_=xr[i][:, sg, :])
            for i in range(2):
                nc.vector.bn_aggr(out=mvc_t[:, i, :], in_=bst_t[:, i, :, :])
                # stat = (mean_c, E[x^2]_c = var_c + mean_c^2)
                nc.vector.tensor_copy(out=stat_t[:, i, 0:1], in_=mvc_t[:, i, 0:1])
                nc.vector.scalar_tensor_tensor(
                    out=stat_t[:, i, 1:2], in0=mvc_t[:, i, 0:1], scalar=mvc_t[:, i, 0:1],
                    in1=mvc_t[:, i, 1:2], op0=AluOp.mult, op1=AluOp.add)

            for i in range(2):
                p16 = ps_misc.tile([16, 2], F32, tag="misc", name="p16")
                nc.tensor.matmul(p16[:], gmat_t[:], stat_t[:, i, :], start=True, stop=True)
                nc.vector.tensor_copy(out=s16_t[:, i, :], in_=p16[:])
                p128 = ps_misc.tile([P, 2], F32, tag="misc", name="p128")
                # gmatT carries the 1/GSIZE scale (host-side) -> (mean_g, E2_g)
                nc.tensor.matmul(p128[:], gmatT_t[:], s16_t[:, i, :], start=True, stop=True)
                nc.vector.tensor_copy(out=mv_t[:, i, :], in_=p128[:])
            # wide views across halves: mean/e2 strided [128, 2]
            mean2 = mv_t[:, :, 0]
            e22 = mv_t[:, :, 1]
            # t1 = var = E2 - mean^2
            nc.vector.tensor_mul(t1_t[:], mean2, mean2)
            nc.vector.tensor_sub(t1_t[:], e22, t1_t[:])
            # sr = sqrt(var + eps)
            nc.scalar.activation(out=sr_t[:], in_=t1_t[:],
                                 func=Act.Sqrt, bias=eps_t[:, 0:1], scale=1.0)
            # ve = var + eps
            nc.vector.tensor_scalar_add(ve_t[:], t1_t[:], EPS)
            nc.vector.reciprocal(out=r0_t[:], in_=sr_t[:])
            # one Newton step: rn = r0 * (1.5 - 0.5 * ve * r0^2)
            nc.vector.tensor_mul(t2_t[:], r0_t[:], r0_t[:])
            nc.vector.tensor_mul(t3_t[:], ve_t[:], t2_t[:])
            nc.vector.tensor_scalar(out=t3_t[:], in0=t3_t[:], scalar1=-0.5, scalar2=1.5,
                                    op0=AluOp.mult, op1=AluOp.add)
            nc.vector.tensor_mul(rn_t[:], r0_t[:], t3_t[:])
            nc.vector.tensor_mul(A_t[:], rn_t[:], gw_t[:])
            nc.vector.tensor_scalar_mul(Aq_t[:], A_t[:], SQ2 / 16.0)
            # nB = mean * A - gn_b   (= -B)
            nc.vector.tensor_mul(nB_t[:], mean2, A_t[:])
            nc.vector.tensor_sub(nB_t[:], nB_t[:], gb_t[:])

            # ---- fused projection weights ----
            for i in range(2):
                nc.scalar.activation(out=wr["q"][:, i, :], in_=wraw["q"][:, i, :],
                                     func=Act.Copy, scale=Aq_t[:, i:i + 1])
                nc.scalar.activation(out=wr["k"][:, i, :], in_=wraw["k"][:, i, :],
                                     func=Act.Copy, scale=A_t[:, i:i + 1])
                nc.scalar.activation(out=wr["v"][:, i, :], in_=wraw["v"][:, i, :],
                                     func=Act.Copy, scale=A_t[:, i:i + 1])

            # ---- bias folds ----
            nc.vector.tensor_scalar_mul(bqs_t[:], bq_t[:], 1.0 / 16.0)
            nc.vector.tensor_scalar_mul(bps_t[:], bp_t[:], RS2)
            for j in range(2):
                jj = slice(j * P, (j + 1) * P)
                pf = ps_misc.tile([P, 1], F32, tag="misc")
                for i in range(2):
                    nc.tensor.matmul(pf[:], wraw["q"][:, i, jj], nB_t[:, i:i + 1],
                                     start=(i == 0), stop=(i == 1))
                # biasq = (bq - foldq) / 16
                nc.vector.scalar_tensor_tensor(
                    out=biasq_t[:, j:j + 1], in0=pf[:], scalar=-1.0 / 16.0,
                    in1=bqs_t[:, j:j + 1], op0=AluOp.mult, op1=AluOp.add)
                pv = ps_misc.tile([P, 1], F32, tag="misc")
                for i in range(2):
                    nc.tensor.matmul(pv[:], wraw["v"][:, i, jj], nB_t[:, i:i + 1],
                                     start=(i == 0), stop=(i == 1))
                # bv' = bv - foldv
                nc.vector.scalar_tensor_tensor(
                    out=bvp_t[:, j:j + 1], in0=pv[:], scalar=-1.0,
                    in1=bv_t[:, j:j + 1], op0=AluOp.mult, op1=AluOp.add)
            for j in range(2):
                jj = slice(j * P, (j + 1) * P)
                pb = ps_misc.tile([P, 1], F32, tag="misc")
                for i in range(2):
                    nc.tensor.matmul(pb[:], wp_t[:, i, jj].bitcast(F32), bvp_t[:, i:i + 1],
                                     start=(i == 0), stop=(i == 1))
                # beta = (bp + foldp) / sqrt(2)
                nc.vector.scalar_tensor_tensor(
                    out=beta_t[:, j:j + 1], in0=pb[:], scalar=RS2,
                    in1=bps_t[:, j:j + 1], op0=AluOp.mult, op1=AluOp.add)

            # ---- K / V^T / Q projections ----
            for j in range(2):
                jj = slice(j * P, (j + 1) * P)
                for mc in range(8):
                    mm = slice(mc * 512, (mc + 1) * 512)
                    pk = ps_misc.tile([P, 512], F32, tag="misc")
                    for i in range(2):
                        nc.tensor.matmul(pk[:], wr["k"][:, i, jj], xf_t[:, i, mm],
                                         start=(i == 0), stop=(i == 1))
                    nc.scalar.activation(out=k_t[:, j, mm], in_=pk[:], func=Act.Copy)
            for mt in range(32):
                mm = slice(mt * P, (mt + 1) * P)
                pv2 = ps_misc.tile([P, C], F32, tag="misc")
                for i in range(2):
                    nc.tensor.matmul(pv2[:], xf_t[:, i, mm], wr["v"][:, i, :],
                                     start=(i == 0), stop=(i == 1))
                nc.vector.tensor_copy(out=vt_t[:, mt, :], in_=pv2[:])
            for j in range(2):
                jj = slice(j * P, (j + 1) * P)
                for nck in range(4):
                    nn = slice(nck * 512, (nck + 1) * 512)
                    pq = ps_misc.tile([P, 512], F32, tag="misc")
                    for i in range(2):
                        nc.tensor.matmul(pq[:], wr["q"][:, i, jj], xqs_t[:, i, nn],
                                         start=(i == 0), stop=(i == 1))
                    nc.vector.tensor_scalar_add(q_t[:, j, nn], pq[:], biasq_t[:, j:j + 1])

            # ---- attention, 4 chunks of 512 query columns ----
            douts = dout.rearrange("(j p) n -> p j n", p=P)
            for nt in range(4):
                nn = slice(nt * 512, (nt + 1) * 512)
                av = ps_av.tile([P, 1024], F32, tag="av")
                db = ps_db.tile([1, 512], F32, tag="db")
                s_tiles = {}

                def s_mm(mt):
                    st = ps_s.tile([P, 512], F32, tag="s")
                    for i in range(2):
                        nc.tensor.matmul(
                            st[:], k_t[:, i, mt * P:(mt + 1) * P], q_t[:, i, nn],
                            start=(i == 0), stop=(i == 1))
                    s_tiles[mt] = st

                s_mm(0)
                s_mm(1)
                for mt in range(32):
                    e = expp.tile([P, 512], F32R, tag="e")
                    nc.scalar.activation(out=e[:], in_=s_tiles.pop(mt)[:], func=Act.Exp)
                    if mt + 2 < 32:
                        s_mm(mt + 2)
                    first, last = mt == 0, mt == 31
                    nc.tensor.matmul(av[:, 0:512], vt_t[:, mt, 0:P], e[:],
                                     start=first, stop=last)
                    nc.tensor.matmul(av[:, 512:1024], vt_t[:, mt, P:C], e[:],
                                     start=first, stop=last)
                    nc.tensor.matmul(db[:], ones_t[:], e[:], start=first, stop=last)

                # Rb = 1 / (sqrt(2) * denom), broadcast across partitions by the
                # all-ones matmul already
                ds1 = rbp.tile([1, 512], F32, tag="ds1")
                nc.vector.tensor_scalar_mul(ds1[:], db[:], SQ2)
                dsb = rbp.tile([P, 512], F32, tag="dsb")
                nc.gpsimd.partition_broadcast(dsb[:], ds1[:])
                rb = rbp.tile([P, 512], F32, tag="rb")
                rsc = rbp.tile([P, 512], F32, tag="rsc")
                nc.vector.reciprocal_approx_accurate(out=rb[:], in_=dsb[:], scratch=rsc[:])

                avs = []
                for i in range(2):
                    a = avp.tile([P, 512], F32R, name="avs", tag="avs")
                    nc.vector.tensor_copy(out=a[:], in_=av[:, i * 512:(i + 1) * 512])
                    avs.append(a)

                for j in range(2):
                    jj = slice(j * P, (j + 1) * P)
                    pj = ps_misc.tile([P, 512], F32, tag="misc")
                    for i in range(2):
                        nc.tensor.matmul(pj[:], wp_t[:, i, jj], avs[i][:],
                                         start=(i == 0), stop=(i == 1))
                    t = outp.tile([P, 512], F32, tag="t")
                    nc.vector.tensor_mul(t[:], pj[:], rb[:])
                    o = outp.tile([P, 512], F32, tag="o")
                    nc.vector.scalar_tensor_tensor(
                        out=o[:], in0=t[:], scalar=beta_t[:, j:j + 1],
                        in1=xqs_t[:, j, nn].bitcast(F32),
                        op0=AluOp.add, op1=AluOp.add)
                    nc.sync.dma_start(out=douts[:, j, nn], in_=o[:])

    nc.compile()
    return nc


_NC = None


def _get_nc():
    global _NC
    if _NC is None:
        _NC = _build()
    return _NC


def _host_inputs(x, gn_w, gn_b, Wq, bq, Wk, bk, Wv, bv, Wp, bp):
    x = np.asarray(x, dtype=np.float32).reshape(B, C, HW)
    gmat = np.zeros((P, 16), dtype=np.float32)
    for p in range(P):
        gmat[p, p // GSIZE] = 1.0
    gmatT = np.ascontiguousarray(gmat.T / GSIZE)
    ones = np.ones((P, 1), dtype=np.float32)
    common = {
        "wq": np.ascontiguousarray(Wq, dtype=np.float32),
        "wk": np.ascontiguousarray(Wk, dtype=np.float32),
        "wv": np.ascontiguousarray(Wv, dtype=np.float32),
        "wp": np.ascontiguousarray(Wp, dtype=np.float32),
        "gnw": np.ascontiguousarray(gn_w, dtype=np.float32),
        "gnb": np.ascontiguousarray(gn_b, dtype=np.float32),
        "bq": np.ascontiguousarray(bq, dtype=np.float32),
        "bv": np.ascontiguousarray(bv, dtype=np.float32),
        "bp": np.ascontiguousarray(bp, dtype=np.float32),
        "gmat": gmat,
        "gmatT": gmatT,
        "ones": ones,
    }
    in_maps = []
    for core in range(8):
        b, qh = core // 2, core % 2
        xb = np.ascontiguousarray(x[b])
        xq = np.ascontiguousarray(x[b][:, qh * NH:(qh + 1) * NH])
        in_maps.append({"xf": xb, "xq": xq, **common})
    return in_maps


def kernel(x, gn_w, gn_b, Wq, bq, Wk, bk, Wv, bv, Wp, bp, _trace=False):
    nc = _get_nc()
    in_maps = _host_inputs(x, gn_w, gn_b, Wq, bq, Wk, bk, Wv, bv, Wp, bp)
    res = bass_utils.run_bass_kernel_spmd(nc, in_maps, core_ids=list(range(8)),
                                          trace=_trace)
    out = np.empty((B, C, HW), dtype=np.float32)
    for core in range(8):
        b, qh = core // 2, core % 2
        out[b][:, qh * NH:(qh + 1) * NH] = res.results[core]["out"]
    if _trace:
        kernel.last_results = res
    return out.reshape(B, C, 64, 64)
